# revision 13
# baseline (speedup 1.0000x reference)
"""Chunked cross-attention (retrieval KNN) Trainium2 Bass kernel.

Problem shapes: x [2048, 1024], neighbours [32, 2, 512, 1024],
Wq/Wk/Wv/Wo [1024, 1024]; 64-token chunks, 2 neighbours x 512 tokens,
16 heads x 64 head-dim; softmax over the QUERY axis (source quirk).

Distribution: data-parallel over the 31 "main" chunks across 8 cores
(cores 0-6: 4 chunks, core 7: 3 chunks + a duplicated dummy), weights
replicated. The degenerate last-token chunk (single query => softmax
over one element => uniform weights) reduces to
(0.5 * sum_kr neighbours[31]) @ Wv.T @ Wo.T and is computed on-device
in a small epilogue block (only core 7's copy is used).

On-chip dataflow per chunk (all layouts pre-transposed on host so the
contraction dim lands on SBUF partitions):
  kkT[l, kr] = sum_d WkT[d, l] * ctxT[d, kr]     (f32r matmuls, N=512)
  vv[kr, l]  = sum_d ctxT[d, kr]^T * WvT[d, l]   (f32r matmuls, N=512)
  qT[l, q]   = sum_d WqT[d, l] * attT[d, q]      (f32r, N=256)
  scores[r, q] per (k,h) = kkT^T q               (bf16 matmuls, N=64)
  softmax over q (free dim): exp -> per-(h,r) row-sum -> reciprocal
  o[q, l] += w'[r, q]^T vv[r, l] over (k, r)     (bf16, N=64)
  z[q, d] = sum_l oT[l, q] * WoT[l, d]           (f32r, N=512; 0.5 & mean
                                                  over k folded into WoT)
"""

import os
import numpy as np
from contextlib import ExitStack

import concourse.bass as bass
import concourse.tile as tile
from concourse import bacc, mybir, masks
from concourse import bass_utils

F32 = mybir.dt.float32
F32R = mybir.dt.float32r
BF16 = mybir.dt.bfloat16
EXP = mybir.ActivationFunctionType.Exp
AX_X = mybir.AxisListType.X
MULT = mybir.AluOpType.mult

N, M, K, R, D = 2048, 64, 2, 512, 1024
H, L = 16, 1024
LCH = N // M          # 32 chunks
DH = L // H           # 64
SCALE = 1.0 / (D ** 0.5)
NCORES = 8
NSLOT = 4             # chunk slots per core
DT = 8                # 128-row tiles along the d (contraction) dim

# chunk assignment: cores 0-6 -> 4 chunks each (0..27), core 7 -> 28,29,30 + dup(30)
CORE_CHUNKS = [[4 * c + i for i in range(4)] for c in range(7)] + [[28, 29, 30, 30]]

_nc_cache = None
KDBG = int(os.environ.get("KDBG", "0"))
KSUB = int(os.environ.get("KSUB", "4"))


def _build_program():
    nc = bacc.Bacc("TRN2", target_bir_lowering=False, debug=False, enable_asserts=False)
    ctxT = nc.dram_tensor("ctxT", [NSLOT, DT, 128, 1024], F32R, kind="ExternalInput").ap()
    attT = nc.dram_tensor("attT", [DT, 128, NSLOT * M], F32R, kind="ExternalInput").ap()
    wqT = nc.dram_tensor("wqT", [DT, 128, 1024], F32R, kind="ExternalInput").ap()
    wkT = nc.dram_tensor("wkT", [DT, 128, 1024], F32R, kind="ExternalInput").ap()
    wvT = nc.dram_tensor("wvT", [DT, 128, 1024], F32R, kind="ExternalInput").ap()
    woT = nc.dram_tensor("woT", [DT, 128, 1024], F32R, kind="ExternalInput").ap()
    c31T = nc.dram_tensor("ctx31T", [DT, 128, 1024], F32R, kind="ExternalInput").ap()
    zout = nc.dram_tensor("z", [NSLOT, M, 1024], F32, kind="ExternalOutput").ap()
    z31out = nc.dram_tensor("z31", [1, 1024], F32, kind="ExternalOutput").ap()

    with tile.TileContext(nc) as tc, ExitStack() as ctx:
        def pool(name, bufs, space=bass.MemorySpace.SBUF):
            return ctx.enter_context(tc.tile_pool(name=name, bufs=bufs, space=space))

        wkp = pool("wk", 8)
        wvp = pool("wv", 8)
        woqp = pool("woq", 8)          # WqT first, then reused for WoT
        ctxp = pool("ctx", 8)
        qtp = pool("qt", 16)
        kkp = pool("kk", 9)
        vvp = pool("vv", 9)
        wrp = pool("wr", 2)
        wsp = pool("ws", 16)
        dnp = pool("dn", 4)
        rcp = pool("rc", 4)
        osbp = pool("osb", 1)
        otp = pool("ot", 1)
        zsbp = pool("zsb", 1)
        idp = pool("idp", 1)
        s31p = pool("s31", 1)
        vstp = pool("vst", 1)
        mmps = pool("mmps", 3, space=bass.MemorySpace.PSUM)
        scps = pool("scps", 3, space=bass.MemorySpace.PSUM)
        ops = pool("ops", 1, space=bass.MemorySpace.PSUM)

        ident = idp.tile([128, 128], F32, tag="id")
        masks.make_identity(nc, ident[:])

        wq_sb, wk_sb, wv_sb, att_sb = [], [], [], []
        for d in range(DT):
            t = woqp.tile([128, 1024], F32R, tag="woq", name=f"wq{d}")
            nc.sync.dma_start(t[:], wqT[d])
            wq_sb.append(t)
        for d in range(DT):
            t = wkp.tile([128, 1024], F32R, tag="wk", name=f"wk{d}")
            nc.sync.dma_start(t[:], wkT[d])
            wk_sb.append(t)
        for d in range(DT):
            t = wvp.tile([128, 1024], F32R, tag="wv", name=f"wv{d}")
            nc.sync.dma_start(t[:], wvT[d])
            wv_sb.append(t)
        for d in range(DT):
            t = wsp.tile([128, NSLOT * M], F32R, tag="ws", name=f"att{d}")
            nc.sync.dma_start(t[:], attT[d])
            att_sb.append(t)

        # qT[l, (slot, q)] for all 4 slots at once. Two zero-padded
        # variants per l-tile (even heads live in partitions 0-63, odd in
        # 64-127; the other half is zeroed) so every score matmul contracts
        # over the full 128 partitions at base partition 0 -- consecutive
        # PE matmuls with differing base partitions hard-fault the exec unit.
        qt_e, qt_o = [], []
        for lt in range(DT):
            ps = mmps.tile([128, NSLOT * M], F32, tag="mm", name=f"qtps{lt}")
            for d in range(DT):
                nc.tensor.matmul(ps[:],
                                 wq_sb[d][:, lt * 128:(lt + 1) * 128],
                                 att_sb[d][:],
                                 start=(d == 0), stop=(d == DT - 1))
            qe = qtp.tile([128, NSLOT * M], BF16, tag="qt", name=f"qte{lt}")
            nc.vector.memset(qe[64:128, :], 0.0)
            nc.scalar.copy(qe[0:64, :], ps[0:64, :])
            qt_e.append(qe)
            qo = qtp.tile([128, NSLOT * M], BF16, tag="qt", name=f"qto{lt}")
            nc.vector.memset(qo[0:64, :], 0.0)
            nc.scalar.copy(qo[64:128, :], ps[64:128, :])
            qt_o.append(qo)

        # WoT into the woq slots (after qT no longer needs WqT)
        wo_sb = []
        for d in range(DT):
            t = woqp.tile([128, 1024], F32R, tag="woq", name=f"wo{d}")
            nc.sync.dma_start(t[:], woT[d])
            wo_sb.append(t)

        for s in range(NSLOT):
            ctx_sb = []
            for d in range(DT):
                t = ctxp.tile([128, 1024], F32R, tag="ctx", name=f"ctx{s}_{d}")
                nc.sync.dma_start(t[:], ctxT[s, d])
                ctx_sb.append(t)

            if KDBG >= 2:
                z_sb = zsbp.tile([M, 1024], F32, tag="zsb", name=f"zsb{s}")
                nc.vector.memset(z_sb[:], 0.0)
                nc.sync.dma_start(zout[s], z_sb[:])
            if KDBG >= 5:
                continue
            # kkT[l, kr] (bf16 for the score matmuls)
            kk_sb = []
            for lt in range(DT):
                kt = kkp.tile([128, 1024], BF16, tag="kk", name=f"kk{s}_{lt}")
                for half in range(2):
                    ps = mmps.tile([128, 512], F32, tag="mm", name=f"kkps{s}_{lt}_{half}")
                    for d in range(DT):
                        nc.tensor.matmul(ps[:],
                                         wk_sb[d][:, lt * 128:(lt + 1) * 128],
                                         ctx_sb[d][:, half * 512:(half + 1) * 512],
                                         start=(d == 0), stop=(d == DT - 1))
                    nc.scalar.copy(kt[:, half * 512:(half + 1) * 512], ps[:])
                kk_sb.append(kt)

            # vv[kr, l] (bf16 for the o matmuls)
            vv_sb = []
            for rt2 in range(8):
                vt = vvp.tile([128, 1024], BF16, tag="vv", name=f"vv{s}_{rt2}")
                for half in range(2):
                    ps = mmps.tile([128, 512], F32, tag="mm", name=f"vvps{s}_{rt2}_{half}")
                    for d in range(DT):
                        nc.tensor.matmul(ps[:],
                                         ctx_sb[d][:, rt2 * 128:(rt2 + 1) * 128],
                                         wv_sb[d][:, half * 512:(half + 1) * 512],
                                         start=(d == 0), stop=(d == DT - 1))
                    nc.scalar.copy(vt[:, half * 512:(half + 1) * 512], ps[:])
                vv_sb.append(vt)

            if KDBG >= 4:
                continue
            # scores -> exp -> row-sum (over q, free dim) -> normalize
            w_sb = {}
            for k in range(2):
                for rt in range(4):
                    for hh in range(2):
                        sps = scps.tile([128, 512], F32, tag="sc", name=f"sc{s}_{k}{rt}{hh}")
                        for hi in range(8):
                            h = hh * 8 + hi
                            lt = h // 2
                            qt = qt_e[lt] if h % 2 == 0 else qt_o[lt]
                            nc.tensor.matmul(
                                sps[:, hi * 64:(hi + 1) * 64],
                                kk_sb[lt][:, k * 512 + rt * 128:k * 512 + (rt + 1) * 128],
                                qt[:, s * M:(s + 1) * M],
                                start=True, stop=True)
                        wr = wrp.tile([128, 512], BF16, tag="wr", name=f"wr{s}_{k}{rt}{hh}")
                        if KSUB >= 1:
                            nc.scalar.activation(wr[:], sps[:], EXP)
                        else:
                            nc.vector.tensor_copy(wr[:], sps[:])
                        dn = dnp.tile([128, 8], F32, tag="dn", name=f"dn{s}_{k}{rt}{hh}")
                        if KSUB >= 2:
                            nc.vector.reduce_sum(
                                dn[:], wr[:].rearrange("p (h q) -> p h q", h=8), axis=AX_X)
                        rc = rcp.tile([128, 8], F32, tag="rc", name=f"rc{s}_{k}{rt}{hh}")
                        if KSUB >= 3:
                            nc.vector.reciprocal(rc[:], dn[:])
                        ws = wsp.tile([128, 512], BF16, tag="ws", name=f"ws{s}_{k}{rt}{hh}")
                        if KSUB >= 4:
                            nc.vector.tensor_tensor(
                                ws[:].rearrange("p (h q) -> p h q", h=8),
                                wr[:].rearrange("p (h q) -> p h q", h=8),
                                rc[:].unsqueeze(2).broadcast_to([128, 8, 64]),
                                op=MULT)
                        else:
                            nc.vector.tensor_copy(ws[:], wr[:])
                        w_sb[(k, rt, hh)] = ws

            if KDBG >= 3:
                continue
            # o[q, l] accumulated over (k, rt); per-head groups kept
            # consecutive so PSUM zero-region groups never interleave
            o_ps = ops.tile([M, 1024], F32, tag="o", name=f"ops{s}")
            for hh in range(2):
                for hi in range(8):
                    h = hh * 8 + hi
                    n = 0
                    for k in range(2):
                        for rt in range(4):
                            nc.tensor.matmul(
                                o_ps[0:M, h * 64:(h + 1) * 64],
                                w_sb[(k, rt, hh)][:, hi * 64:(hi + 1) * 64],
                                vv_sb[k * 4 + rt][:, h * 64:(h + 1) * 64],
                                start=(n == 0), stop=(n == 7))
                            n += 1

            if KDBG >= 2:
                continue
            o_sb = osbp.tile([M, 1024], F32, tag="osb", name=f"osb{s}")
            nc.scalar.copy(o_sb[:], o_ps[:])
            ot = otp.tile([128, 512], F32R, tag="ot", name=f"ot{s}")
            for lt in range(DT):
                tps = mmps.tile([128, M], F32, tag="mm", name=f"trps{s}_{lt}")
                nc.tensor.transpose(tps[:], o_sb[:, lt * 128:(lt + 1) * 128], ident[0:M, 0:M])
                nc.scalar.copy(ot[:, lt * 64:(lt + 1) * 64], tps[:])

            z_sb = zsbp.tile([M, 1024], F32, tag="zsb", name=f"zsb{s}")
            for half in range(2):
                ps = mmps.tile([M, 512], F32, tag="mm", name=f"zps{s}_{half}")
                for lt in range(DT):
                    nc.tensor.matmul(ps[:],
                                     ot[:, lt * 64:(lt + 1) * 64],
                                     wo_sb[lt][:, half * 512:(half + 1) * 512],
                                     start=(lt == 0), stop=(lt == DT - 1))
                nc.vector.tensor_copy(z_sb[:, half * 512:(half + 1) * 512], ps[:])
            nc.sync.dma_start(zout[s], z_sb[:])

        # ---- last-token chunk: z31 = (sum_kr ctx31) @ WvT @ WoT ----
        if KDBG >= 1:
            z31_sb = zsbp.tile([M, 1024], F32, tag="zsb", name="z31sb")
            nc.vector.memset(z31_sb[0:1, :], 0.0)
            nc.sync.dma_start(z31out[:], z31_sb[0:1, :])
        else:
            c31_sb = []
            for d in range(DT):
                t = ctxp.tile([128, 1024], F32R, tag="ctx", name=f"c31_{d}")
                nc.sync.dma_start(t[:], c31T[d])
                c31_sb.append(t)
            s31f = s31p.tile([128, DT], F32, tag="s31f")
            for d in range(DT):
                nc.vector.reduce_sum(s31f[:, d:d + 1], c31_sb[d][:].bitcast(F32), axis=AX_X)
            # duplicated columns: f32r matmuls need a moving free dim >= 2
            s31 = s31p.tile([128, 2 * DT], F32R, tag="s31r")
            nc.scalar.copy(s31[:].rearrange("p (d two) -> p d two", two=2),
                           s31f[:].unsqueeze(2).broadcast_to([128, DT, 2]))
            vst = vstp.tile([128, DT], F32R, tag="vst")
            for lt in range(DT):
                ps = mmps.tile([128, 2], F32, tag="mm", name=f"vstps{lt}")
                for d in range(DT):
                    nc.tensor.matmul(ps[:],
                                     wv_sb[d][:, lt * 128:(lt + 1) * 128],
                                     s31[:, 2 * d:2 * d + 2],
                                     start=(d == 0), stop=(d == DT - 1))
                nc.scalar.copy(vst[:, lt:lt + 1], ps[:, 0:1])
            z31_sb = zsbp.tile([M, 1024], F32, tag="zsb", name="z31sb")
            for half in range(2):
                ps = mmps.tile([1, 512], F32, tag="mm", name=f"z31ps{half}")
                for lt in range(DT):
                    nc.tensor.matmul(ps[:],
                                     vst[:, lt:lt + 1],
                                     wo_sb[lt][:, half * 512:(half + 1) * 512],
                                     start=(lt == 0), stop=(lt == DT - 1))
                nc.vector.tensor_copy(z31_sb[0:1, half * 512:(half + 1) * 512], ps[:])
            nc.sync.dma_start(z31out[:], z31_sb[0:1, :])

    nc.compile()
    return nc


def _get_program():
    global _nc_cache
    if _nc_cache is None:
        _nc_cache = _build_program()
    return _nc_cache


def _prep_inputs(x, neighbours, Wq, Wk, Wv, Wo):
    x = np.ascontiguousarray(np.asarray(x, dtype=np.float32))
    neighbours = np.ascontiguousarray(np.asarray(neighbours, dtype=np.float32))
    wqT = np.ascontiguousarray((np.asarray(Wq, np.float32).T * SCALE)).reshape(DT, 128, 1024)
    wkT = np.ascontiguousarray(np.asarray(Wk, np.float32).T).reshape(DT, 128, 1024)
    wvT = np.ascontiguousarray(np.asarray(Wv, np.float32).T).reshape(DT, 128, 1024)
    woT = np.ascontiguousarray(np.asarray(Wo, np.float32).T * 0.5).reshape(DT, 128, 1024)

    zeros31 = np.zeros((DT, 128, 1024), np.float32)
    c31 = np.ascontiguousarray(neighbours[31].reshape(1024, 1024).T).reshape(DT, 128, 1024)

    in_maps = []
    for c in range(NCORES):
        chunks = CORE_CHUNKS[c]
        att = np.concatenate(
            [x[M - 1 + M * u: M - 1 + M * (u + 1)] for u in chunks], axis=0)  # [256, 1024]
        attT = np.ascontiguousarray(att.T).reshape(DT, 128, NSLOT * M)
        ctxT = np.stack(
            [np.ascontiguousarray(neighbours[u].reshape(1024, 1024).T).reshape(DT, 128, 1024)
             for u in chunks])
        in_maps.append({
            "ctxT": ctxT,
            "attT": attT,
            "wqT": wqT, "wkT": wkT, "wvT": wvT, "woT": woT,
            "ctx31T": c31 if c == NCORES - 1 else zeros31,
        })
    return x, in_maps


def _assemble(x, results):
    out = np.empty((N, D), np.float32)
    out[:M - 1] = x[:M - 1]
    done = set()
    for c in range(NCORES):
        for si, u in enumerate(CORE_CHUNKS[c]):
            if u in done:
                continue
            done.add(u)
            out[M - 1 + M * u: M - 1 + M * (u + 1)] = results[c]["z"][si]
    out[N - 1] = results[NCORES - 1]["z31"][0]
    return out


def _run(x, in_maps, trace=False):
    nc = _get_program()
    res = bass_utils.run_bass_kernel_spmd(nc, in_maps, core_ids=list(range(NCORES)),
                                          trace=trace)
    return res


def kernel(x, neighbours, Wq, Wk, Wv, Wo):
    x, in_maps = _prep_inputs(x, neighbours, Wq, Wk, Wv, Wo)
    res = _run(x, in_maps, trace=False)
    return _assemble(x, res.results)


def kernel_timed(x, neighbours, Wq, Wk, Wv, Wo):
    """Same as kernel() but also returns the profiled HW execution time (ns)."""
    x, in_maps = _prep_inputs(x, neighbours, Wq, Wk, Wv, Wo)
    res = _run(x, in_maps, trace=True)
    return _assemble(x, res.results), res.exec_time_ns
